# revision 3
# baseline (speedup 1.0000x reference)
"""GCNConv Trainium2 kernel: out = (segsum_{dst}(x[src]*norm[src]) @ W) * norm[dst] + bias.

Distribution: dst nodes are bin-packed (snake over degree-sorted order) into
800 blocks of 128 slots, 100 blocks per core, equalizing per-block edge
counts. The halo exchange of the sharding hint — "all-to-all of scaled
source features" — is materialized host-side: each core's input is its
edges' scaled source features bf16(x[src]*norm[src]), already grouped by
destination block and padded to TB tiles of 128 edges. The device streams
these affinely at full HBM bandwidth (no per-edge descriptors) and does all
the arithmetic: one-hot selection matrices S[e, d] = (iota == dstl) built
on DVE/GpSimd, segment-sum via bf16 matmuls accumulating in fp32 PSUM,
projection through W, then norm[dst] scale + bias.

This replaces the previous dma_gather design whose Q7 descriptor generation
(~8 ns/edge, 96% GpSimd occupancy in the trace) was the bottleneck.
"""

import numpy as np
import ml_dtypes

N = 100000
C = 128
NC_ = 8
NBLK = 100                 # dst blocks per core (128 slots each)
NBINS = NC_ * NBLK         # 800
BF16 = ml_dtypes.bfloat16

_prog_cache = {}


def _build_program(TB):
    import concourse.bacc as bacc
    import concourse.mybir as mybir
    import concourse.tile as tile
    from contextlib import ExitStack

    f32 = mybir.dt.float32
    bf16 = mybir.dt.bfloat16

    nc = bacc.Bacc("TRN2", target_bir_lowering=False, debug=False)
    msgs_d = nc.dram_tensor("msgs", [128, NBLK * TB * C], bf16, kind="ExternalInput")
    dstl_d = nc.dram_tensor("dstl", [128, NBLK * TB], f32, kind="ExternalInput")
    ndst_d = nc.dram_tensor("ndst", [128, NBLK], f32, kind="ExternalInput")
    w_d = nc.dram_tensor("w", [C, C], bf16, kind="ExternalInput")
    biasb_d = nc.dram_tensor("biasb", [128, C], f32, kind="ExternalInput")
    iota_d = nc.dram_tensor("iota", [128, 128], bf16, kind="ExternalInput")
    out_d = nc.dram_tensor("out", [NBLK * 128, C], f32, kind="ExternalOutput")

    with tile.TileContext(nc) as tc, ExitStack() as ctx:
        const = ctx.enter_context(tc.tile_pool(name="const", bufs=1))
        dstl_sb = const.tile([128, NBLK * TB], f32)
        nc.sync.dma_start(dstl_sb[:], dstl_d.ap()[:])
        ndst_sb = const.tile([128, NBLK], f32)
        nc.sync.dma_start(ndst_sb[:], ndst_d.ap()[:])
        w_sb = const.tile([C, C], bf16)
        nc.sync.dma_start(w_sb[:], w_d.ap()[:])
        biasb_sb = const.tile([128, C], f32)
        nc.sync.dma_start(biasb_sb[:], biasb_d.ap()[:])
        iota_sb = const.tile([128, 128], bf16)
        nc.sync.dma_start(iota_sb[:], iota_d.ap()[:])

        mpool = ctx.enter_context(tc.tile_pool(name="msgs", bufs=4))
        spool = ctx.enter_context(tc.tile_pool(name="sel", bufs=12))
        apool = ctx.enter_context(tc.tile_pool(name="aggT", bufs=3))
        opool = ctx.enter_context(tc.tile_pool(name="outt", bufs=3))
        accp = ctx.enter_context(tc.tile_pool(name="acc", bufs=4, space="PSUM"))
        projp = ctx.enter_context(tc.tile_pool(name="proj", bufs=2, space="PSUM"))

        for b in range(NBLK):
            m = mpool.tile([128, TB * C], bf16)
            nc.sync.dma_start(m[:], msgs_d.ap()[:, b * TB * C:(b + 1) * TB * C])
            acc = accp.tile([128, 128], f32)
            for u in range(TB):
                col = b * TB + u
                S = spool.tile([128, 128], bf16)
                eng = nc.vector if (u % 2 == 0) else nc.gpsimd
                eng.tensor_scalar(
                    out=S[:],
                    in0=iota_sb[:],
                    scalar1=dstl_sb[:, col:col + 1],
                    scalar2=None,
                    op0=mybir.AluOpType.is_equal,
                )
                nc.tensor.matmul(
                    out=acc[:],
                    lhsT=m[:, u * C:(u + 1) * C],
                    rhs=S[:],
                    start=(u == 0),
                    stop=(u == TB - 1),
                )
            aggT = apool.tile([128, 128], bf16)
            nc.scalar.copy(aggT[:], acc[:])
            proj = projp.tile([128, 128], f32)
            nc.tensor.matmul(out=proj[:], lhsT=aggT[:], rhs=w_sb[:], start=True, stop=True)
            outt = opool.tile([128, C], f32)
            nc.vector.scalar_tensor_tensor(
                out=outt[:],
                in0=proj[:],
                scalar=ndst_sb[:, b:b + 1],
                in1=biasb_sb[:],
                op0=mybir.AluOpType.mult,
                op1=mybir.AluOpType.add,
            )
            nc.sync.dma_start(out_d.ap()[b * 128:(b + 1) * 128, :], outt[:])
    nc.compile()
    return nc


def _preprocess(x, norm, weight, bias, edge_src, edge_dst):
    src = np.asarray(edge_src).astype(np.int64, copy=False).ravel()
    dst = np.asarray(edge_dst).astype(np.int64, copy=False).ravel()
    E = src.size
    normf = np.asarray(norm, dtype=np.float32).ravel()

    # --- dst -> (core, block, local-row) via degree-balanced snake packing ---
    deg = np.bincount(dst, minlength=N)
    order_d = np.argsort(-deg, kind="stable")
    i = np.arange(N, dtype=np.int64)
    rnd, col = i // NBINS, i % NBINS
    bin_pos = np.where(rnd % 2 == 0, col, NBINS - 1 - col)
    bin_of = np.empty(N, np.int64)
    loc_of = np.empty(N, np.int64)
    bin_of[order_d] = bin_pos
    loc_of[order_d] = rnd
    core_of = bin_of // NBLK
    blk_of = bin_of % NBLK

    # --- edge slotting: group by (core, block), sequential slots ---
    e_bin = bin_of[dst]
    cnt = np.bincount(e_bin, minlength=NBINS)
    TB = int(np.ceil(cnt.max() / 128))
    cap = TB * 128
    order_e = np.argsort(e_bin, kind="stable")
    starts = np.concatenate([[0], np.cumsum(cnt)[:-1]])
    rank = np.arange(E, dtype=np.int64) - starts[e_bin[order_e]]
    slot = e_bin[order_e] * cap + rank          # flat (bin, slot)

    # --- scaled source features, bf16 (the halo-exchange payload) ---
    xs = (np.asarray(x, np.float32) * normf[:, None]).astype(BF16)
    msgs = np.zeros((NBINS * cap, C), BF16)
    msgs[slot] = xs[src[order_e]]
    dstl = np.full(NBINS * cap, -1.0, np.float32)
    dstl[slot] = loc_of[dst[order_e]].astype(np.float32)

    # device layout: [core][p, b, u, c] with edge slot s = u*128 + p
    msgs = msgs.reshape(NC_, NBLK, TB, 128, C).transpose(0, 3, 1, 2, 4)
    msgs = np.ascontiguousarray(msgs.reshape(NC_, 128, NBLK * TB * C))
    dstl = dstl.reshape(NC_, NBLK, TB, 128).transpose(0, 3, 1, 2)
    dstl = np.ascontiguousarray(dstl.reshape(NC_, 128, NBLK * TB))

    ndst = np.zeros((NC_, 128, NBLK), np.float32)
    ndst[core_of, loc_of, blk_of] = normf

    w = np.asarray(weight, np.float32).astype(BF16)
    biasb = np.broadcast_to(np.asarray(bias, np.float32), (128, C)).copy()
    iota = np.broadcast_to(np.arange(128, dtype=np.float32), (128, 128)).astype(BF16)

    in_maps = [{
        "msgs": msgs[k],
        "dstl": dstl[k],
        "ndst": np.ascontiguousarray(ndst[k]),
        "w": w,
        "biasb": biasb,
        "iota": iota,
    } for k in range(NC_)]

    # output row of each dst node in the concatenated per-core outputs
    row_of = core_of * (NBLK * 128) + blk_of * 128 + loc_of
    return TB, in_maps, row_of


def _run(inputs, trace=False, trace_kwargs=None):
    from concourse.bass_utils import run_bass_kernel_spmd

    TB, in_maps, row_of = _preprocess(**inputs)
    if TB not in _prog_cache:
        _prog_cache[TB] = _build_program(TB)
    nc = _prog_cache[TB]
    kw = {}
    if trace:
        kw["trace"] = True
        if trace_kwargs:
            kw["trace_kwargs"] = trace_kwargs
    res = run_bass_kernel_spmd(nc, in_maps, core_ids=list(range(NC_)), **kw)
    big = np.concatenate([res.results[k]["out"] for k in range(NC_)], axis=0)
    return big[row_of], res


def kernel(**inputs):
    out, _ = _run(inputs, trace=False)
    return out


# revision 4
# speedup vs baseline: 5.2989x; 5.2989x over previous
"""GCNConv Trainium2 kernel: out = (segsum_{dst}(x[src]*norm[src]) @ W) * norm[dst] + bias.

Distribution: dst nodes are bin-packed (snake over degree-sorted order) into
800 blocks of 128 slots, 100 blocks per core, equalizing per-block edge
counts. The halo exchange of the sharding hint — "all-to-all of scaled
source features" — is materialized host-side: each core's input is its
edges' scaled source features bf16(x[src]*norm[src]), already grouped by
destination block and padded to TB tiles of 128 edges. The device streams
these affinely at full HBM bandwidth (no per-edge descriptors) and does all
the arithmetic: one-hot selection matrices S[e, d] = (iota == dstl) built
on DVE/GpSimd, segment-sum via bf16 matmuls accumulating in fp32 PSUM,
projection through W, then norm[dst] scale + bias.

This replaces the previous dma_gather design whose Q7 descriptor generation
(~8 ns/edge, 96% GpSimd occupancy in the trace) was the bottleneck.
"""

import numpy as np
import ml_dtypes

N = 100000
C = 128
NC_ = 8
NBLK = 100                 # dst blocks per core (128 slots each)
NBINS = NC_ * NBLK         # 800
BF16 = ml_dtypes.bfloat16

_prog_cache = {}


def _build_program(TB):
    import concourse.bacc as bacc
    import concourse.mybir as mybir
    import concourse.tile as tile
    from contextlib import ExitStack

    f32 = mybir.dt.float32
    bf16 = mybir.dt.bfloat16

    nc = bacc.Bacc("TRN2", target_bir_lowering=False, debug=False)
    msgs_d = nc.dram_tensor("msgs", [128, NBLK * TB * C], bf16, kind="ExternalInput")
    dstl_d = nc.dram_tensor("dstl", [128, NBLK * TB], bf16, kind="ExternalInput")
    ndst_d = nc.dram_tensor("ndst", [128, NBLK], f32, kind="ExternalInput")
    w_d = nc.dram_tensor("w", [C, C], bf16, kind="ExternalInput")
    biasb_d = nc.dram_tensor("biasb", [128, C], f32, kind="ExternalInput")
    iota_d = nc.dram_tensor("iota", [128, 128], bf16, kind="ExternalInput")
    out_d = nc.dram_tensor("out", [NBLK * 128, C], f32, kind="ExternalOutput")

    with tile.TileContext(nc) as tc, ExitStack() as ctx:
        const = ctx.enter_context(tc.tile_pool(name="const", bufs=1))
        dstl_sb = const.tile([128, NBLK * TB], bf16)
        nc.sync.dma_start(dstl_sb[:], dstl_d.ap()[:])
        ndst_sb = const.tile([128, NBLK], f32)
        nc.sync.dma_start(ndst_sb[:], ndst_d.ap()[:])
        w_sb = const.tile([C, C], bf16)
        nc.sync.dma_start(w_sb[:], w_d.ap()[:])
        biasb_sb = const.tile([128, C], f32)
        nc.sync.dma_start(biasb_sb[:], biasb_d.ap()[:])
        iota_sb = const.tile([128, 128], bf16)
        nc.sync.dma_start(iota_sb[:], iota_d.ap()[:])

        mpool = ctx.enter_context(tc.tile_pool(name="msgs", bufs=4))
        spool = ctx.enter_context(tc.tile_pool(name="sel", bufs=3))
        apool = ctx.enter_context(tc.tile_pool(name="aggT", bufs=3))
        opool = ctx.enter_context(tc.tile_pool(name="outt", bufs=3))
        accp = ctx.enter_context(tc.tile_pool(name="acc", bufs=4, space="PSUM"))
        projp = ctx.enter_context(tc.tile_pool(name="proj", bufs=2, space="PSUM"))

        for b in range(NBLK):
            m = mpool.tile([128, TB * C], bf16)
            nc.sync.dma_start(m[:], msgs_d.ap()[:, b * TB * C:(b + 1) * TB * C])
            acc = accp.tile([128, 128], f32)
            S = spool.tile([128, TB * 128], bf16)
            i0 = iota_sb[:].rearrange("p (o f) -> p o f", o=1).broadcast_to([128, TB, 128])
            i1 = dstl_sb[:, b * TB:(b + 1) * TB].rearrange(
                "p (t o) -> p t o", o=1).broadcast_to([128, TB, 128])
            nc.vector.tensor_tensor(
                out=S[:].rearrange("p (t f) -> p t f", f=128),
                in0=i0, in1=i1, op=mybir.AluOpType.is_equal,
            )
            for u in range(TB):
                nc.tensor.matmul(
                    out=acc[:],
                    lhsT=m[:, u * C:(u + 1) * C],
                    rhs=S[:, u * 128:(u + 1) * 128],
                    start=(u == 0),
                    stop=(u == TB - 1),
                )
            aggT = apool.tile([128, 128], bf16)
            nc.scalar.copy(aggT[:], acc[:])
            proj = projp.tile([128, 128], f32)
            nc.tensor.matmul(out=proj[:], lhsT=aggT[:], rhs=w_sb[:], start=True, stop=True)
            outt = opool.tile([128, C], f32)
            nc.vector.scalar_tensor_tensor(
                out=outt[:],
                in0=proj[:],
                scalar=ndst_sb[:, b:b + 1],
                in1=biasb_sb[:],
                op0=mybir.AluOpType.mult,
                op1=mybir.AluOpType.add,
            )
            nc.sync.dma_start(out_d.ap()[b * 128:(b + 1) * 128, :], outt[:])
    nc.compile()
    return nc


def _preprocess(x, norm, weight, bias, edge_src, edge_dst):
    src = np.asarray(edge_src).astype(np.int64, copy=False).ravel()
    dst = np.asarray(edge_dst).astype(np.int64, copy=False).ravel()
    E = src.size
    normf = np.asarray(norm, dtype=np.float32).ravel()

    # --- dst -> (core, block, local-row) via degree-balanced snake packing ---
    deg = np.bincount(dst, minlength=N)
    order_d = np.argsort(-deg, kind="stable")
    i = np.arange(N, dtype=np.int64)
    rnd, col = i // NBINS, i % NBINS
    bin_pos = np.where(rnd % 2 == 0, col, NBINS - 1 - col)
    bin_of = np.empty(N, np.int64)
    loc_of = np.empty(N, np.int64)
    bin_of[order_d] = bin_pos
    loc_of[order_d] = rnd
    core_of = bin_of // NBLK
    blk_of = bin_of % NBLK

    # --- edge slotting: group by (core, block), sequential slots ---
    e_bin = bin_of[dst]
    cnt = np.bincount(e_bin, minlength=NBINS)
    TB = int(np.ceil(cnt.max() / 128))
    cap = TB * 128
    order_e = np.argsort(e_bin, kind="stable")
    starts = np.concatenate([[0], np.cumsum(cnt)[:-1]])
    rank = np.arange(E, dtype=np.int64) - starts[e_bin[order_e]]
    slot = e_bin[order_e] * cap + rank          # flat (bin, slot)

    # --- scaled source features, bf16 (the halo-exchange payload) ---
    xs = (np.asarray(x, np.float32) * normf[:, None]).astype(BF16)
    msgs = np.zeros((NBINS * cap, C), BF16)
    msgs[slot] = xs[src[order_e]]
    dstl = np.full(NBINS * cap, -1.0, np.float32)
    dstl[slot] = loc_of[dst[order_e]].astype(np.float32)

    # device layout: [core][p, b, u, c] with edge slot s = u*128 + p
    msgs = msgs.reshape(NC_, NBLK, TB, 128, C).transpose(0, 3, 1, 2, 4)
    msgs = np.ascontiguousarray(msgs.reshape(NC_, 128, NBLK * TB * C))
    dstl = dstl.reshape(NC_, NBLK, TB, 128).transpose(0, 3, 1, 2)
    dstl = np.ascontiguousarray(dstl.reshape(NC_, 128, NBLK * TB).astype(BF16))

    ndst = np.zeros((NC_, 128, NBLK), np.float32)
    ndst[core_of, loc_of, blk_of] = normf

    w = np.asarray(weight, np.float32).astype(BF16)
    biasb = np.broadcast_to(np.asarray(bias, np.float32), (128, C)).copy()
    iota = np.broadcast_to(np.arange(128, dtype=np.float32), (128, 128)).astype(BF16)

    in_maps = [{
        "msgs": msgs[k],
        "dstl": dstl[k],
        "ndst": np.ascontiguousarray(ndst[k]),
        "w": w,
        "biasb": biasb,
        "iota": iota,
    } for k in range(NC_)]

    # output row of each dst node in the concatenated per-core outputs
    row_of = core_of * (NBLK * 128) + blk_of * 128 + loc_of
    return TB, in_maps, row_of


def _run(inputs, trace=False, trace_kwargs=None):
    from concourse.bass_utils import run_bass_kernel_spmd

    TB, in_maps, row_of = _preprocess(**inputs)
    if TB not in _prog_cache:
        _prog_cache[TB] = _build_program(TB)
    nc = _prog_cache[TB]
    kw = {}
    if trace:
        kw["trace"] = True
        if trace_kwargs:
            kw["trace_kwargs"] = trace_kwargs
    res = run_bass_kernel_spmd(nc, in_maps, core_ids=list(range(NC_)), **kw)
    big = np.concatenate([res.results[k]["out"] for k in range(NC_)], axis=0)
    return big[row_of], res


def kernel(**inputs):
    out, _ = _run(inputs, trace=False)
    return out


# revision 5
# speedup vs baseline: 7.2782x; 1.3735x over previous
"""GCNConv Trainium2 kernel: out = (segsum_{dst}(x[src]*norm[src]) @ W) * norm[dst] + bias.

Distribution: dst nodes are bin-packed (snake over degree-sorted order) into
800 blocks of 128 slots, 100 blocks per core, equalizing per-block edge
counts. The halo exchange of the sharding hint — "all-to-all of scaled
source features" — is materialized host-side: each core's input is its
edges' scaled source features bf16(x[src]*norm[src]), already grouped by
destination block and padded to TB tiles of 128 edges. The device streams
these affinely at full HBM bandwidth (no per-edge descriptors) and does all
the arithmetic: one-hot selection matrices S[e, d] = (iota == dstl) built
as one wide DVE tensor_tensor per block pair (stride-0 broadcast APs),
segment-sum via bf16 matmuls accumulating in fp32 PSUM, projection through
W with bias folded in as a rank-1 matmul (invnorm[d] * bias[c]), and the
final norm[dst] scale on the Scalar engine (per-partition activation
scale), keeping DVE dedicated to S.

Engine budget per core (measured): DVE S-build ~2.2us per 2048-col block
(port-bound), PE ~1800 bf16 matmuls, DMA ~52 MB streamed. The previous
dma_gather design burned 96% GpSimd on Q7 descriptor generation (~8 ns/edge).
"""

import numpy as np
import ml_dtypes

N = 100000
C = 128
NC_ = 8
NBLK = 100                 # dst blocks per core (128 slots each)
NBINS = NC_ * NBLK         # 800
PAIR = 2                   # blocks per S-build / msgs DMA
BF16 = ml_dtypes.bfloat16

_prog_cache = {}


def _build_program(TB):
    import concourse.bacc as bacc
    import concourse.mybir as mybir
    import concourse.tile as tile
    from contextlib import ExitStack

    f32 = mybir.dt.float32
    bf16 = mybir.dt.bfloat16

    nc = bacc.Bacc("TRN2", target_bir_lowering=False, debug=False)
    msgs_d = nc.dram_tensor("msgs", [128, NBLK * TB * C], bf16, kind="ExternalInput")
    dstl_d = nc.dram_tensor("dstl", [128, NBLK * TB], bf16, kind="ExternalInput")
    ndst_d = nc.dram_tensor("ndst", [128, NBLK], f32, kind="ExternalInput")
    invnb_d = nc.dram_tensor("invnb", [1, NBLK * 128], f32, kind="ExternalInput")
    w_d = nc.dram_tensor("w", [C, C], bf16, kind="ExternalInput")
    biasr_d = nc.dram_tensor("biasr", [1, C], f32, kind="ExternalInput")
    iota_d = nc.dram_tensor("iota", [128, 128], bf16, kind="ExternalInput")
    out_d = nc.dram_tensor("out", [NBLK * 128, C], f32, kind="ExternalOutput")

    with tile.TileContext(nc) as tc, ExitStack() as ctx:
        const = ctx.enter_context(tc.tile_pool(name="const", bufs=1))
        dstl_sb = const.tile([128, NBLK * TB], bf16)
        nc.sync.dma_start(dstl_sb[:], dstl_d.ap()[:])
        ndst_sb = const.tile([128, NBLK], f32)
        nc.sync.dma_start(ndst_sb[:], ndst_d.ap()[:])
        invnb_sb = const.tile([1, NBLK * 128], f32)
        nc.sync.dma_start(invnb_sb[:], invnb_d.ap()[:])
        w_sb = const.tile([C, C], bf16)
        nc.sync.dma_start(w_sb[:], w_d.ap()[:])
        biasr_sb = const.tile([1, C], f32)
        nc.sync.dma_start(biasr_sb[:], biasr_d.ap()[:])
        iota_sb = const.tile([128, 128], bf16)
        nc.sync.dma_start(iota_sb[:], iota_d.ap()[:])

        mpool = ctx.enter_context(tc.tile_pool(name="msgs", bufs=4))
        spool = ctx.enter_context(tc.tile_pool(name="sel", bufs=4))
        apool = ctx.enter_context(tc.tile_pool(name="aggT", bufs=3))
        opool = ctx.enter_context(tc.tile_pool(name="outt", bufs=4))
        accp = ctx.enter_context(tc.tile_pool(name="acc", bufs=4, space="PSUM"))
        projp = ctx.enter_context(tc.tile_pool(name="proj", bufs=2, space="PSUM"))

        for g in range(NBLK // PAIR):
            b0 = g * PAIR
            m = mpool.tile([128, PAIR * TB * C], bf16)
            nc.sync.dma_start(m[:], msgs_d.ap()[:, b0 * TB * C:(b0 + PAIR) * TB * C])
            S = spool.tile([128, PAIR * TB * 128], bf16)
            i0 = iota_sb[:].rearrange("p (o f) -> p o f", o=1).broadcast_to(
                [128, PAIR * TB, 128])
            i1 = dstl_sb[:, b0 * TB:(b0 + PAIR) * TB].rearrange(
                "p (t o) -> p t o", o=1).broadcast_to([128, PAIR * TB, 128])
            nc.vector.tensor_tensor(
                out=S[:].rearrange("p (t f) -> p t f", f=128),
                in0=i0, in1=i1, op=mybir.AluOpType.is_equal,
            )
            for j in range(PAIR):
                b = b0 + j
                acc = accp.tile([128, 128], f32)
                for u in range(TB):
                    t = j * TB + u
                    nc.tensor.matmul(
                        out=acc[:],
                        lhsT=m[:, t * C:(t + 1) * C],
                        rhs=S[:, t * 128:(t + 1) * 128],
                        start=(u == 0),
                        stop=(u == TB - 1),
                    )
                aggT = apool.tile([128, 128], bf16)
                nc.scalar.copy(aggT[:], acc[:])
                proj = projp.tile([128, 128], f32)
                nc.tensor.matmul(out=proj[:], lhsT=aggT[:], rhs=w_sb[:],
                                 start=True, stop=False)
                nc.tensor.matmul(out=proj[:],
                                 lhsT=invnb_sb[:, b * 128:(b + 1) * 128],
                                 rhs=biasr_sb[:], start=False, stop=True)
                outt = opool.tile([128, C], f32)
                nc.scalar.activation(
                    out=outt[:], in_=proj[:],
                    func=mybir.ActivationFunctionType.Copy,
                    scale=ndst_sb[:, b:b + 1],
                )
                nc.sync.dma_start(out_d.ap()[b * 128:(b + 1) * 128, :], outt[:])
    nc.compile()
    return nc


def _preprocess(x, norm, weight, bias, edge_src, edge_dst):
    src = np.asarray(edge_src).astype(np.int64, copy=False).ravel()
    dst = np.asarray(edge_dst).astype(np.int64, copy=False).ravel()
    E = src.size
    normf = np.asarray(norm, dtype=np.float32).ravel()

    # --- dst -> (core, block, local-row) via degree-balanced snake packing ---
    deg = np.bincount(dst, minlength=N)
    order_d = np.argsort(-deg, kind="stable")
    i = np.arange(N, dtype=np.int64)
    rnd, col = i // NBINS, i % NBINS
    bin_pos = np.where(rnd % 2 == 0, col, NBINS - 1 - col)
    bin_of = np.empty(N, np.int64)
    loc_of = np.empty(N, np.int64)
    bin_of[order_d] = bin_pos
    loc_of[order_d] = rnd
    core_of = bin_of // NBLK
    blk_of = bin_of % NBLK

    # --- edge slotting: group by (core, block), sequential slots ---
    e_bin = bin_of[dst]
    cnt = np.bincount(e_bin, minlength=NBINS)
    TB = int(np.ceil(cnt.max() / 128))
    cap = TB * 128
    order_e = np.argsort(e_bin, kind="stable")
    starts = np.concatenate([[0], np.cumsum(cnt)[:-1]])
    rank = np.arange(E, dtype=np.int64) - starts[e_bin[order_e]]
    slot = e_bin[order_e] * cap + rank          # flat (bin, slot)

    # --- scaled source features, bf16 (the halo-exchange payload) ---
    xs = (np.asarray(x, np.float32) * normf[:, None]).astype(BF16)
    msgs = np.zeros((NBINS * cap, C), BF16)
    msgs[slot] = xs[src[order_e]]
    dstl = np.full(NBINS * cap, -1.0, np.float32)
    dstl[slot] = loc_of[dst[order_e]].astype(np.float32)

    # device layout: [core][p, b, u, c] with edge slot s = u*128 + p
    msgs = msgs.reshape(NC_, NBLK, TB, 128, C).transpose(0, 3, 1, 2, 4)
    msgs = np.ascontiguousarray(msgs.reshape(NC_, 128, NBLK * TB * C))
    dstl = dstl.reshape(NC_, NBLK, TB, 128).transpose(0, 3, 1, 2)
    dstl = np.ascontiguousarray(dstl.reshape(NC_, 128, NBLK * TB).astype(BF16))

    ndst = np.ones((NC_, 128, NBLK), np.float32)
    ndst[core_of, loc_of, blk_of] = normf
    invnb = (1.0 / ndst).transpose(0, 2, 1).reshape(NC_, 1, NBLK * 128)

    w = np.asarray(weight, np.float32).astype(BF16)
    biasr = np.asarray(bias, np.float32).reshape(1, C)
    iota = np.broadcast_to(np.arange(128, dtype=np.float32), (128, 128)).astype(BF16)

    in_maps = [{
        "msgs": msgs[k],
        "dstl": dstl[k],
        "ndst": np.ascontiguousarray(ndst[k]),
        "invnb": np.ascontiguousarray(invnb[k]),
        "w": w,
        "biasr": biasr,
        "iota": iota,
    } for k in range(NC_)]

    # output row of each dst node in the concatenated per-core outputs
    row_of = core_of * (NBLK * 128) + blk_of * 128 + loc_of
    return TB, in_maps, row_of


def _run(inputs, trace=False, trace_kwargs=None):
    from concourse.bass_utils import run_bass_kernel_spmd

    TB, in_maps, row_of = _preprocess(**inputs)
    if TB not in _prog_cache:
        _prog_cache[TB] = _build_program(TB)
    nc = _prog_cache[TB]
    kw = {}
    if trace:
        kw["trace"] = True
        if trace_kwargs:
            kw["trace_kwargs"] = trace_kwargs
    res = run_bass_kernel_spmd(nc, in_maps, core_ids=list(range(NC_)), **kw)
    big = np.concatenate([res.results[k]["out"] for k in range(NC_)], axis=0)
    return big[row_of], res


def kernel(**inputs):
    out, _ = _run(inputs, trace=False)
    return out
